# revision 19
# baseline (speedup 1.0000x reference)
"""Trainium2 Bass kernel for CustomWindowMHA (sparse window+dilated attention).

Sharding: 8 cores = 2 batches x 4 head-groups (4 heads each). Each core
computes QKV projection for its heads, masked attention, and a partial
output projection against its slice of wo's columns; the host sums the 4
partials per batch.

Layout choices (all matmuls bf16 with fp32 PSUM accumulation):
  - Q^T, K^T [dh, S] computed via transposed projection (lhsT = qkv cols).
  - Scores computed transposed: ST[j, q] = K^T.T @ Q^T, so softmax exp runs
    on ACT straight out of PSUM and the PV matmul needs no transposes.
  - V kept token-major [S, dh] with a fused ones-column, so the PV matmul
    (lhsT = [V | 1], rhs = exp(ST)) yields out^T[dh, q] AND the softmax
    denominator row L[q] in one accumulation.
  - Final projection consumes out^T directly as lhsT (no transposes).
  - Mask (window + dilated, depends only on delta = qtile - jtile) is a
    precomputed 0/1 bf16 table multiplied in on DVE after exp.
"""

import sys

sys.path.insert(0, "/opt/trn_rl_repo")

import numpy as np
import ml_dtypes

import concourse.bass as bass
import concourse.mybir as mybir
import concourse.tile as tile
from concourse.vector_clock import ScopedClock
from concourse.bass_utils import run_bass_kernel_spmd

BF16 = mybir.dt.bfloat16
F32 = mybir.dt.float32

B, S, D = 2, 2048, 1024
H, DH = 16, 64
WINDOW, DILATION = 128, 4
P = 128
NT = S // P          # 16 token tiles
KT = D // P          # 8 contraction tiles over D
HPC = 4              # heads per core
QC = 512             # q-chunk width
NQC = S // QC        # 4 q-chunks
NDELTA = 19          # mask table deltas -3..15
NMASKB = 24          # 19 delta masks + Ab [5,128] + Bb [5,512] rank-5 bias
BIAS = 200.0         # additive-mask magnitude: exp(0.125*-200) ~ 1.4e-11


class _TileContext(tile.TileContext):
    """Kernel-tail Drain gets one wait per live proc, but this walrus build
    allows only a single sync wait on SP Drain — split across drains."""

    def _drain_and_barrier(self, tick_clock, wait_clock):
        drain_inst = self.nc.sync.drain()
        wait_clock.add_sem_waits(
            drain_inst.ins, ScopedClock({None: tick_clock.global_clock})
        )
        si = drain_inst.ins.sync_info
        if si is not None and len(si.on_wait) > 1:
            waits = list(si.on_wait)
            si.on_wait[:] = waits[:1]
            for w in waits[1:]:
                d2 = self.nc.sync.drain()
                si2 = d2.ins.sync_info
                if si2 is None:
                    d2.ins.sync_info = mybir.SyncInfo(on_wait=[w], on_update=[])
                else:
                    si2.on_wait[:] = [w]

        self.nc.all_engine_barrier()
        assert self.sems is not None
        popped = self.nc._tile_sem_poison_stack.pop()
        assert popped is self._sem_poison
        self.nc.clear_and_free_semaphores(list(self.sems.allocated().values()))
        self.nc.all_engine_barrier()


def _split_sync_waits(nc):
    """This walrus build allows only one sync-wait slot on several ISA
    structs. Rewrite the scheduled BIR so every instruction carries at most
    one wait: extra waits move onto same-engine NoOps inserted just before
    (same engine queue => executes in order => semantics preserved)."""
    cnt = 0
    for fn in nc.m.functions:
        for blk in fn.blocks:
            new_insts = []
            for inst in blk.instructions:
                si = inst.sync_info
                if si is not None and si.on_wait and len(si.on_wait) > 1:
                    waits = list(si.on_wait)
                    si.on_wait[:] = waits[-1:]
                    for w in waits[:-1]:
                        cnt += 1
                        nop = mybir.InstNoOp(
                            name=f"waitsplit-{cnt}",
                            engine=inst.engine,
                            ins=[],
                            outs=[],
                            sync_info=mybir.SyncInfo(on_wait=[w], on_update=[]),
                        )
                        new_insts.append(nop)
                new_insts.append(inst)
            blk.instructions[:] = new_insts
    return cnt


def _mask_table() -> np.ndarray:
    """[128, NDELTA*128] bf16; block d (delta = d-3) holds mask[sj, sq] for
    score-block ST[jt, qt] with qt - jt = delta: offset = 128*delta + sq - sj."""
    sj = np.arange(P)[:, None]
    sq = np.arange(P)[None, :]
    out = np.zeros((P, NMASKB * P), dtype=ml_dtypes.bfloat16)
    for d in range(NDELTA):
        delta = d - 3
        off = 128 * delta + sq - sj
        valid = ((off >= 0) & (off <= WINDOW - 1)) | (
            (off >= WINDOW + DILATION) & ((off - WINDOW) % DILATION == 0)
        )
        out[:, d * P : (d + 1) * P] = valid.astype(ml_dtypes.bfloat16)
    # rank-5 additive mask for pure-dilated blocks (delta >= 2):
    # valid iff sj % 4 == sq % 4; bias = Ab.T @ Bb = 0 valid / -BIAS invalid.
    r = np.arange(4)[:, None]
    ab = np.zeros((P, P), dtype=ml_dtypes.bfloat16)
    ab[0:4, :] = (BIAS * (np.arange(P)[None, :] % 4 == r)).astype(ml_dtypes.bfloat16)
    ab[4, :] = BIAS
    out[:, NDELTA * P : (NDELTA + 1) * P] = ab
    bb = np.zeros((P, 4 * P), dtype=ml_dtypes.bfloat16)
    bb[0:4, :] = (np.arange(4 * P)[None, :] % 4 == r).astype(ml_dtypes.bfloat16)
    bb[4, :] = -1.0
    out[:, (NDELTA + 1) * P :] = bb
    return out


def _build_program(repeat: int = 1):
    nc = bass.Bass("TRN2", target_bir_lowering=False, debug=False)

    xt_d = nc.declare_dram_parameter("xt", [D, S], BF16, isOutput=False)
    qkvt_d = nc.declare_dram_parameter("qkvt", [D, 3 * HPC * DH], BF16, isOutput=False)
    wot_d = nc.declare_dram_parameter("wot", [HPC * DH, D], BF16, isOutput=False)
    mask_d = nc.declare_dram_parameter("mask", [P, NMASKB * P], BF16, isOutput=False)
    y_d = nc.declare_dram_parameter("y", [S, D], F32, isOutput=True)

    W3 = 3 * HPC * DH  # 768 qkvt columns per k-tile

    with _TileContext(nc) as tc:
        with (
            tc.tile_pool(name="const", bufs=1) as cpool,
            tc.tile_pool(name="work", bufs=2) as wpool,
            tc.tile_pool(name="psum", bufs=2, space="PSUM") as pspool,
        ):
            # ---- persistent SBUF tensors ----
            xt_sb = cpool.tile([P, KT * S], BF16, tag="xt")
            qkvt_sb = cpool.tile([P, KT * W3], BF16, tag="qkvt")
            mask_sb = cpool.tile([P, NMASKB * P], BF16, tag="mask")
            wot_sb = cpool.tile([P, 2 * D], BF16, tag="wot")
            qkt_sb = cpool.tile([P, 4 * S], BF16, tag="qkt")    # QT (mt 0,1), KT (mt 2,3)
            vaug_sb = cpool.tile([P, NT, HPC, DH + 1], BF16, tag="vaug")
            outt_sb = cpool.tile([P, 2 * S], BF16, tag="outt")  # out^T, ct-major
            ones1_sb = cpool.tile([1, 64], mybir.dt.float16, tag="ones1")

            nc.gpsimd.memset(vaug_sb[:], 1.0)
            nc.gpsimd.memset(ones1_sb[:], 1.0)

            for _rep in range(repeat):
                # ---- input DMA ----
                for kt in range(KT):
                    nc.sync.dma_start(
                        out=xt_sb[:, kt * S : (kt + 1) * S],
                        in_=xt_d[kt * P : (kt + 1) * P, :],
                    )
                    nc.sync.dma_start(
                        out=qkvt_sb[:, kt * W3 : (kt + 1) * W3],
                        in_=qkvt_d[kt * P : (kt + 1) * P, :],
                    )
                nc.sync.dma_start(out=mask_sb[:], in_=mask_d[:])
                for ct in range(2):
                    nc.sync.dma_start(
                        out=wot_sb[:, ct * D : (ct + 1) * D],
                        in_=wot_d[ct * P : (ct + 1) * P, :],
                    )

                # ---- QT / KT projection (transposed) ----
                # mt 0,1 = Q channels 0:128 / 128:256; mt 2,3 = K channels.
                for mt in range(4):
                    coloff = (0, 128, 256, 384)[mt]
                    for cc in range(NQC):
                        ps = pspool.tile([P, QC], F32, tag="ps512")
                        for kt in range(KT):
                            nc.tensor.matmul(
                                ps[:],
                                lhsT=qkvt_sb[:, kt * W3 + coloff : kt * W3 + coloff + P],
                                rhs=xt_sb[:, kt * S + cc * QC : kt * S + (cc + 1) * QC],
                                start=(kt == 0),
                                stop=(kt == KT - 1),
                            )
                        nc.scalar.copy(
                            qkt_sb[:, mt * S + cc * QC : mt * S + (cc + 1) * QC], ps[:]
                        )

                # ---- V projection (token-major) into vaug ----
                for nt in range(NT):
                    ps = pspool.tile([P, HPC * DH], F32, tag="ps512")
                    for kt in range(KT):
                        nc.tensor.matmul(
                            ps[:],
                            lhsT=xt_sb[:, kt * S + nt * P : kt * S + (nt + 1) * P],
                            rhs=qkvt_sb[:, kt * W3 + 512 : kt * W3 + 768],
                            start=(kt == 0),
                            stop=(kt == KT - 1),
                        )
                    for h in range(HPC):
                        nc.vector.tensor_copy(
                            vaug_sb[:, nt, h, 0:DH], ps[:, h * DH : (h + 1) * DH]
                        )

                # ---- attention + output projection, per q-chunk ----
                for qc in range(NQC):
                    for h in range(HPC):
                        pb = 64 * (h % 2)
                        qoff = (h // 2) * S
                        koff = (2 + h // 2) * S
                        njt = 4 * qc + 4
                        poT = pspool.tile([P, QC], F32, tag="pot")
                        # software pipeline (depth 2): issue ST(jt) two steps
                        # ahead of PV(jt) so the PE queue never stalls on the
                        # ACT exp / DVE mask of the current chunk.
                        ps_t, e_t = {}, {}

                        def _issue_st(jt):
                            pure = (4 * qc - jt) >= 2  # all sub-blocks dilated-only
                            ps = pspool.tile([P, QC], F32, tag="st", bufs=3)
                            nc.tensor.matmul(
                                ps[:],
                                lhsT=qkt_sb[
                                    pb : pb + 64, koff + jt * P : koff + (jt + 1) * P
                                ],
                                rhs=qkt_sb[
                                    pb : pb + 64, qoff + qc * QC : qoff + (qc + 1) * QC
                                ],
                                start=True,
                                stop=not pure,
                            )
                            if pure:
                                # additive rank-5 mask: 0 where sj%4==sq%4 else -BIAS
                                nc.tensor.matmul(
                                    ps[:],
                                    lhsT=mask_sb[0:5, NDELTA * P : (NDELTA + 1) * P],
                                    rhs=mask_sb[
                                        0:5, (NDELTA + 1) * P : (NDELTA + 1) * P + QC
                                    ],
                                    start=False,
                                    stop=True,
                                )
                            ps_t[jt] = ps

                        def _issue_exp(jt):
                            pure = (4 * qc - jt) >= 2
                            e = wpool.tile([P, QC], BF16, tag="e", bufs=6)
                            nc.scalar.activation(
                                e[:],
                                ps_t.pop(jt)[:],
                                mybir.ActivationFunctionType.Exp,
                                scale=0.125,
                            )
                            if not pure:
                                didx = 4 * qc - jt + 3
                                nc.vector.tensor_mul(
                                    e[:], e[:], mask_sb[:, didx * P : didx * P + QC]
                                )
                            e_t[jt] = e

                        def _issue_pv(jt):
                            nc.tensor.matmul(
                                poT[0:65, :],
                                lhsT=vaug_sb[:, jt, h, :],
                                rhs=e_t.pop(jt)[:],
                                start=(jt == 0),
                                stop=(jt == njt - 1),
                            )

                        for step in range(njt + 2):
                            if step < njt:
                                _issue_st(step)
                            if 0 <= step - 1 < njt:
                                _issue_exp(step - 1)
                            if 0 <= step - 2 < njt:
                                _issue_pv(step - 2)
                        # 1/L: reshape the [1,512] L row to [128,4] via DMA so
                        # the DVE reciprocal runs on all lanes (cost ~ free size)
                        lrow = wpool.tile([1, QC], F32, tag="lrow", bufs=2)
                        nc.scalar.copy(lrow[:], poT[64:65, :])
                        r4 = wpool.tile([P, 4], F32, tag="r4", bufs=2)
                        nc.sync.dma_start(
                            out=r4[:],
                            in_=lrow[0:1, :].rearrange("a (p c) -> a p c", p=P),
                        )
                        i4 = wpool.tile([P, 4], F32, tag="i4", bufs=2)
                        nc.vector.reciprocal(i4[:], r4[:])
                        i4h = wpool.tile([P, 4], mybir.dt.float16, tag="i4h", bufs=2)
                        nc.vector.tensor_copy(i4h[:], i4[:])
                        invl16 = wpool.tile([1, QC], mybir.dt.float16, tag="invl16", bufs=2)
                        nc.sync.dma_start(
                            out=invl16[0:1, :].rearrange("a (p c) -> a p c", p=P),
                            in_=i4h[:],
                        )
                        # broadcast invl over 64 partitions via a K=1 ones-matmul
                        ib = pspool.tile([64, QC], F32, tag="ib", bufs=1)
                        nc.tensor.matmul(
                            ib[:],
                            lhsT=ones1_sb[:],
                            rhs=invl16[:],
                            start=True,
                            stop=True,
                        )
                        ibs = wpool.tile([64, QC], F32, tag="ibs", bufs=2)
                        nc.scalar.copy(ibs[:], ib[:])
                        nc.vector.scalar_tensor_tensor(
                            out=outt_sb[
                                pb : pb + 64, qoff + qc * QC : qoff + (qc + 1) * QC
                            ],
                            in0=poT[0:64, :],
                            scalar=1.0,
                            in1=ibs[:],
                            op0=mybir.AluOpType.mult,
                            op1=mybir.AluOpType.mult,
                        )
                    # wo partial projection for the 4 q-tiles of this chunk
                    for qt in range(4 * qc, 4 * qc + 4):
                        ysb = wpool.tile([P, D], F32, tag="ysb", bufs=2)
                        for oc in range(2):
                            yps = pspool.tile([P, QC], F32, tag="ps512")
                            for ct in range(2):
                                nc.tensor.matmul(
                                    yps[:],
                                    lhsT=outt_sb[:, ct * S + qt * P : ct * S + (qt + 1) * P],
                                    rhs=wot_sb[:, ct * D + oc * QC : ct * D + (oc + 1) * QC],
                                    start=(ct == 0),
                                    stop=(ct == 1),
                                )
                            nc.vector.tensor_copy(ysb[:, oc * QC : (oc + 1) * QC], yps[:])
                        nc.sync.dma_start(out=y_d[qt * P : (qt + 1) * P, :], in_=ysb[:])

    _split_sync_waits(nc)
    return nc


_PROGRAMS = {}


def _program(repeat: int = 1):
    if repeat not in _PROGRAMS:
        _PROGRAMS[repeat] = _build_program(repeat)
    return _PROGRAMS[repeat]


def _prep_inputs(x, qkv, wo):
    """Per-core host-side slicing/transposition/casting."""
    mask = _mask_table()
    in_maps = []
    for c in range(8):
        b, hg = c // 4, c % 4
        h0 = HPC * hg
        rows = np.r_[
            h0 * DH : h0 * DH + HPC * DH,
            D + h0 * DH : D + h0 * DH + HPC * DH,
            2 * D + h0 * DH : 2 * D + h0 * DH + HPC * DH,
        ]
        qkvt = np.ascontiguousarray(qkv[rows].T).astype(ml_dtypes.bfloat16)
        xt = np.ascontiguousarray(x[b].T).astype(ml_dtypes.bfloat16)
        wot = np.ascontiguousarray(
            wo[:, h0 * DH : h0 * DH + HPC * DH].T
        ).astype(ml_dtypes.bfloat16)
        in_maps.append({"xt": xt, "qkvt": qkvt, "wot": wot, "mask": mask})
    return in_maps


def kernel(x, qkv, wo, _trace=False, _trace_kwargs=None):
    x = np.asarray(x, dtype=np.float32)
    qkv = np.asarray(qkv, dtype=np.float32)
    wo = np.asarray(wo, dtype=np.float32)

    nc = _program()
    in_maps = _prep_inputs(x, qkv, wo)
    res = run_bass_kernel_spmd(
        nc, in_maps, list(range(8)), trace=_trace, **(_trace_kwargs or {})
    )
    kernel.last_result = res

    y = np.zeros((B, S, D), dtype=np.float32)
    for c in range(8):
        y[c // 4] += res.results[c]["y"]
    return y


# revision 21
# speedup vs baseline: 1.1710x; 1.1710x over previous
"""Trainium2 Bass kernel for CustomWindowMHA (sparse window+dilated attention).

Sharding: 8 cores = 2 batches x 4 head-groups (4 heads each). Each core
computes QKV projection for its heads, masked attention, and a partial
output projection against its slice of wo's columns; the host sums the 4
partials per batch.

Layout choices (all matmuls bf16 with fp32 PSUM accumulation):
  - Q^T, K^T [dh, S] computed via transposed projection (lhsT = qkv cols).
  - Scores computed transposed: ST[j, q] = K^T.T @ Q^T, so softmax exp runs
    on ACT straight out of PSUM and the PV matmul needs no transposes.
  - V kept token-major [S, dh] with a fused ones-column, so the PV matmul
    (lhsT = [V | 1], rhs = exp(ST)) yields out^T[dh, q] AND the softmax
    denominator row L[q] in one accumulation.
  - Final projection consumes out^T directly as lhsT (no transposes).
  - Mask (window + dilated, depends only on delta = qtile - jtile) is a
    precomputed 0/1 bf16 table multiplied in on DVE after exp.
"""

import sys

sys.path.insert(0, "/opt/trn_rl_repo")

import numpy as np
import ml_dtypes

import concourse.bass as bass
import concourse.mybir as mybir
import concourse.tile as tile
from concourse.vector_clock import ScopedClock
from concourse.bass_utils import run_bass_kernel_spmd

BF16 = mybir.dt.bfloat16
F32 = mybir.dt.float32

B, S, D = 2, 2048, 1024
H, DH = 16, 64
WINDOW, DILATION = 128, 4
P = 128
NT = S // P          # 16 token tiles
KT = D // P          # 8 contraction tiles over D
HPC = 4              # heads per core
QC = 512             # q-chunk width
NQC = S // QC        # 4 q-chunks
NDELTA = 19          # mask table deltas -3..15
NMASKB = 24          # 19 delta masks + Ab [5,128] + Bb [5,512] rank-5 bias
BIAS = 200.0         # additive-mask magnitude: exp(0.125*-200) ~ 1.4e-11


class _TileContext(tile.TileContext):
    """Kernel-tail Drain gets one wait per live proc, but this walrus build
    allows only a single sync wait on SP Drain — split across drains."""

    def _drain_and_barrier(self, tick_clock, wait_clock):
        drain_inst = self.nc.sync.drain()
        wait_clock.add_sem_waits(
            drain_inst.ins, ScopedClock({None: tick_clock.global_clock})
        )
        si = drain_inst.ins.sync_info
        if si is not None and len(si.on_wait) > 1:
            waits = list(si.on_wait)
            si.on_wait[:] = waits[:1]
            for w in waits[1:]:
                d2 = self.nc.sync.drain()
                si2 = d2.ins.sync_info
                if si2 is None:
                    d2.ins.sync_info = mybir.SyncInfo(on_wait=[w], on_update=[])
                else:
                    si2.on_wait[:] = [w]

        self.nc.all_engine_barrier()
        assert self.sems is not None
        popped = self.nc._tile_sem_poison_stack.pop()
        assert popped is self._sem_poison
        self.nc.clear_and_free_semaphores(list(self.sems.allocated().values()))
        self.nc.all_engine_barrier()


def _split_sync_waits(nc):
    """This walrus build allows only one sync-wait slot on several ISA
    structs. Rewrite the scheduled BIR so every instruction carries at most
    one wait: extra waits move onto same-engine NoOps inserted just before
    (same engine queue => executes in order => semantics preserved)."""
    cnt = 0
    for fn in nc.m.functions:
        for blk in fn.blocks:
            new_insts = []
            for inst in blk.instructions:
                si = inst.sync_info
                if si is not None and si.on_wait and len(si.on_wait) > 1:
                    waits = list(si.on_wait)
                    si.on_wait[:] = waits[-1:]
                    for w in waits[:-1]:
                        cnt += 1
                        nop = mybir.InstNoOp(
                            name=f"waitsplit-{cnt}",
                            engine=inst.engine,
                            ins=[],
                            outs=[],
                            sync_info=mybir.SyncInfo(on_wait=[w], on_update=[]),
                        )
                        new_insts.append(nop)
                new_insts.append(inst)
            blk.instructions[:] = new_insts
    return cnt


def _mask_table() -> np.ndarray:
    """[128, NDELTA*128] bf16; block d (delta = d-3) holds mask[sj, sq] for
    score-block ST[jt, qt] with qt - jt = delta: offset = 128*delta + sq - sj."""
    sj = np.arange(P)[:, None]
    sq = np.arange(P)[None, :]
    out = np.zeros((P, NMASKB * P), dtype=ml_dtypes.bfloat16)
    for d in range(NDELTA):
        delta = d - 3
        off = 128 * delta + sq - sj
        valid = ((off >= 0) & (off <= WINDOW - 1)) | (
            (off >= WINDOW + DILATION) & ((off - WINDOW) % DILATION == 0)
        )
        out[:, d * P : (d + 1) * P] = valid.astype(ml_dtypes.bfloat16)
    # rank-5 additive mask for pure-dilated blocks (delta >= 2):
    # valid iff sj % 4 == sq % 4; bias = Ab.T @ Bb = 0 valid / -BIAS invalid.
    r = np.arange(4)[:, None]
    ab = np.zeros((P, P), dtype=ml_dtypes.bfloat16)
    ab[0:4, :] = (BIAS * (np.arange(P)[None, :] % 4 == r)).astype(ml_dtypes.bfloat16)
    ab[4, :] = BIAS
    out[:, NDELTA * P : (NDELTA + 1) * P] = ab
    bb = np.zeros((P, 4 * P), dtype=ml_dtypes.bfloat16)
    bb[0:4, :] = (np.arange(4 * P)[None, :] % 4 == r).astype(ml_dtypes.bfloat16)
    bb[4, :] = -1.0
    out[:, (NDELTA + 1) * P :] = bb
    return out


def _build_program(repeat: int = 1):
    nc = bass.Bass("TRN2", target_bir_lowering=False, debug=False)

    xt_d = nc.declare_dram_parameter("xt", [D, S], BF16, isOutput=False)
    qkvt_d = nc.declare_dram_parameter("qkvt", [D, 3 * HPC * DH], BF16, isOutput=False)
    wot_d = nc.declare_dram_parameter("wot", [HPC * DH, D], BF16, isOutput=False)
    mask_d = nc.declare_dram_parameter("mask", [P, NMASKB * P], BF16, isOutput=False)
    y_d = nc.declare_dram_parameter("y", [S, D], F32, isOutput=True)

    W3 = 3 * HPC * DH  # 768 qkvt columns per k-tile

    with _TileContext(nc) as tc:
        with (
            tc.tile_pool(name="const", bufs=1) as cpool,
            tc.tile_pool(name="work", bufs=2) as wpool,
            tc.tile_pool(name="psum", bufs=2, space="PSUM") as pspool,
        ):
            # ---- persistent SBUF tensors ----
            xt_sb = cpool.tile([P, KT * S], BF16, tag="xt")
            qkvt_sb = cpool.tile([P, KT * W3], BF16, tag="qkvt")
            mask_sb = cpool.tile([P, NMASKB * P], BF16, tag="mask")
            wot_sb = cpool.tile([P, 2 * D], BF16, tag="wot")
            qkt_sb = cpool.tile([P, 4 * S], BF16, tag="qkt")    # QT (mt 0,1), KT (mt 2,3)
            vaug_sb = cpool.tile([P, NT, HPC, DH + 1], BF16, tag="vaug")
            outt_sb = cpool.tile([P, 2 * S], BF16, tag="outt")  # out^T, ct-major
            ones1_sb = cpool.tile([1, 64], mybir.dt.float16, tag="ones1")

            nc.gpsimd.memset(vaug_sb[:], 1.0)
            nc.gpsimd.memset(ones1_sb[:], 1.0)

            for _rep in range(repeat):
                # ---- input DMA ----
                for kt in range(KT):
                    nc.sync.dma_start(
                        out=xt_sb[:, kt * S : (kt + 1) * S],
                        in_=xt_d[kt * P : (kt + 1) * P, :],
                    )
                    nc.sync.dma_start(
                        out=qkvt_sb[:, kt * W3 : (kt + 1) * W3],
                        in_=qkvt_d[kt * P : (kt + 1) * P, :],
                    )
                nc.sync.dma_start(out=mask_sb[:], in_=mask_d[:])
                for ct in range(2):
                    nc.sync.dma_start(
                        out=wot_sb[:, ct * D : (ct + 1) * D],
                        in_=wot_d[ct * P : (ct + 1) * P, :],
                    )

                # ---- QT / KT projection (transposed) ----
                # mt 0,1 = Q channels 0:128 / 128:256; mt 2,3 = K channels.
                for mt in range(4):
                    coloff = (0, 128, 256, 384)[mt]
                    for cc in range(NQC):
                        ps = pspool.tile([P, QC], F32, tag="ps512")
                        for kt in range(KT):
                            nc.tensor.matmul(
                                ps[:],
                                lhsT=qkvt_sb[:, kt * W3 + coloff : kt * W3 + coloff + P],
                                rhs=xt_sb[:, kt * S + cc * QC : kt * S + (cc + 1) * QC],
                                start=(kt == 0),
                                stop=(kt == KT - 1),
                            )
                        nc.scalar.copy(
                            qkt_sb[:, mt * S + cc * QC : mt * S + (cc + 1) * QC], ps[:]
                        )

                # ---- V projection (token-major) into vaug ----
                for nt in range(NT):
                    ps = pspool.tile([P, HPC * DH], F32, tag="ps512")
                    for kt in range(KT):
                        nc.tensor.matmul(
                            ps[:],
                            lhsT=xt_sb[:, kt * S + nt * P : kt * S + (nt + 1) * P],
                            rhs=qkvt_sb[:, kt * W3 + 512 : kt * W3 + 768],
                            start=(kt == 0),
                            stop=(kt == KT - 1),
                        )
                    for h in range(HPC):
                        nc.vector.tensor_copy(
                            vaug_sb[:, nt, h, 0:DH], ps[:, h * DH : (h + 1) * DH]
                        )

                # ---- attention + output projection, per q-chunk ----
                def _normalize(h, qc, poT):
                    """outt[h rows, qc] = poT[0:64] / L (row 64), via a
                    DMA-reshaped narrow reciprocal + fp16 ones-matmul bcast."""
                    pb = 64 * (h % 2)
                    qoff = (h // 2) * S
                    lrow = wpool.tile([1, QC], F32, tag="lrow", bufs=2)
                    nc.scalar.copy(lrow[:], poT[64:65, :])
                    r4 = wpool.tile([P, 4], F32, tag="r4", bufs=2)
                    nc.sync.dma_start(
                        out=r4[:],
                        in_=lrow[0:1, :].rearrange("a (p c) -> a p c", p=P),
                    )
                    i4 = wpool.tile([P, 4], F32, tag="i4", bufs=2)
                    nc.vector.reciprocal(i4[:], r4[:])
                    i4h = wpool.tile([P, 4], mybir.dt.float16, tag="i4h", bufs=2)
                    nc.vector.tensor_copy(i4h[:], i4[:])
                    invl16 = wpool.tile([1, QC], mybir.dt.float16, tag="invl16", bufs=2)
                    nc.sync.dma_start(
                        out=invl16[0:1, :].rearrange("a (p c) -> a p c", p=P),
                        in_=i4h[:],
                    )
                    ib = pspool.tile([64, QC], F32, tag="ps512", bufs=2)
                    nc.tensor.matmul(
                        ib[:], lhsT=ones1_sb[:], rhs=invl16[:], start=True, stop=True
                    )
                    ibs = wpool.tile([64, QC], F32, tag="ibs", bufs=2)
                    nc.scalar.copy(ibs[:], ib[:])
                    nc.vector.scalar_tensor_tensor(
                        out=outt_sb[pb : pb + 64, qoff + qc * QC : qoff + (qc + 1) * QC],
                        in0=poT[0:64, :],
                        scalar=1.0,
                        in1=ibs[:],
                        op0=mybir.AluOpType.mult,
                        op1=mybir.AluOpType.mult,
                    )

                for qc in range(NQC):
                    njt = 4 * qc + 4
                    # heads processed in pairs (2hp, 2hp+1): their K=64 ST
                    # matmuls sit in disjoint PE row groups (partitions 0-63 /
                    # 64-127) and run concurrently when adjacent in the queue.
                    for hp in range(2):
                        qoff = hp * S
                        koff = (2 + hp) * S
                        poT_a = pspool.tile([P, QC], F32, tag="pot")
                        poT_b = pspool.tile([P, QC], F32, tag="pot")
                        poTs = [poT_a, poT_b]
                        ps_t, e_t = {}, {}

                        def _issue_st(jt):
                            pair = []
                            for i, pb in enumerate((0, 64)):
                                ps = pspool.tile([P, QC], F32, tag="st", bufs=4)
                                nc.tensor.matmul(
                                    ps[:],
                                    lhsT=qkt_sb[
                                        pb : pb + 64,
                                        koff + jt * P : koff + (jt + 1) * P,
                                    ],
                                    rhs=qkt_sb[
                                        pb : pb + 64,
                                        qoff + qc * QC : qoff + (qc + 1) * QC,
                                    ],
                                    start=True,
                                    stop=True,
                                )
                                pair.append(ps)
                            ps_t[jt] = pair

                        def _issue_exp(jt):
                            pure = (4 * qc - jt) >= 2  # all sub-blocks dilated-only
                            didx = 4 * qc - jt + 3
                            pair = []
                            for i in range(2):
                                e = wpool.tile([P, QC], BF16, tag="e", bufs=6)
                                nc.scalar.activation(
                                    e[:],
                                    ps_t[jt][i][:],
                                    mybir.ActivationFunctionType.Exp,
                                    scale=0.125,
                                )
                                # mask multiply: idle GPSIMD for pure-dilated
                                # chunks, DVE for window/edge chunks
                                eng = nc.gpsimd if pure else nc.vector
                                eng.tensor_mul(
                                    e[:], e[:], mask_sb[:, didx * P : didx * P + QC]
                                )
                                pair.append(e)
                            del ps_t[jt]
                            e_t[jt] = pair

                        def _issue_pv(jt):
                            for i in range(2):
                                nc.tensor.matmul(
                                    poTs[i][0:65, :],
                                    lhsT=vaug_sb[:, jt, 2 * hp + i, :],
                                    rhs=e_t[jt][i][:],
                                    start=(jt == 0),
                                    stop=(jt == njt - 1),
                                )
                            del e_t[jt]

                        for step in range(njt + 2):
                            if step < njt:
                                _issue_st(step)
                            if 0 <= step - 1 < njt:
                                _issue_exp(step - 1)
                            if 0 <= step - 2 < njt:
                                _issue_pv(step - 2)
                        _normalize(2 * hp, qc, poTs[0])
                        _normalize(2 * hp + 1, qc, poTs[1])
                    # wo partial projection for the 4 q-tiles of this chunk
                    for qt in range(4 * qc, 4 * qc + 4):
                        ysb = wpool.tile([P, D], F32, tag="ysb", bufs=2)
                        for oc in range(2):
                            yps = pspool.tile([P, QC], F32, tag="ps512")
                            for ct in range(2):
                                nc.tensor.matmul(
                                    yps[:],
                                    lhsT=outt_sb[:, ct * S + qt * P : ct * S + (qt + 1) * P],
                                    rhs=wot_sb[:, ct * D + oc * QC : ct * D + (oc + 1) * QC],
                                    start=(ct == 0),
                                    stop=(ct == 1),
                                )
                            nc.vector.tensor_copy(ysb[:, oc * QC : (oc + 1) * QC], yps[:])
                        nc.sync.dma_start(out=y_d[qt * P : (qt + 1) * P, :], in_=ysb[:])

    _split_sync_waits(nc)
    return nc


_PROGRAMS = {}


def _program(repeat: int = 1):
    if repeat not in _PROGRAMS:
        _PROGRAMS[repeat] = _build_program(repeat)
    return _PROGRAMS[repeat]


def _prep_inputs(x, qkv, wo):
    """Per-core host-side slicing/transposition/casting."""
    mask = _mask_table()
    in_maps = []
    for c in range(8):
        b, hg = c // 4, c % 4
        h0 = HPC * hg
        rows = np.r_[
            h0 * DH : h0 * DH + HPC * DH,
            D + h0 * DH : D + h0 * DH + HPC * DH,
            2 * D + h0 * DH : 2 * D + h0 * DH + HPC * DH,
        ]
        qkvt = np.ascontiguousarray(qkv[rows].T).astype(ml_dtypes.bfloat16)
        xt = np.ascontiguousarray(x[b].T).astype(ml_dtypes.bfloat16)
        wot = np.ascontiguousarray(
            wo[:, h0 * DH : h0 * DH + HPC * DH].T
        ).astype(ml_dtypes.bfloat16)
        in_maps.append({"xt": xt, "qkvt": qkvt, "wot": wot, "mask": mask})
    return in_maps


def kernel(x, qkv, wo, _trace=False, _trace_kwargs=None):
    x = np.asarray(x, dtype=np.float32)
    qkv = np.asarray(qkv, dtype=np.float32)
    wo = np.asarray(wo, dtype=np.float32)

    nc = _program()
    in_maps = _prep_inputs(x, qkv, wo)
    res = run_bass_kernel_spmd(
        nc, in_maps, list(range(8)), trace=_trace, **(_trace_kwargs or {})
    )
    kernel.last_result = res

    y = np.zeros((B, S, D), dtype=np.float32)
    for c in range(8):
        y[c // 4] += res.results[c]["y"]
    return y


# revision 23
# speedup vs baseline: 1.2847x; 1.0971x over previous
"""Trainium2 Bass kernel for CustomWindowMHA (sparse window+dilated attention).

Sharding: 8 cores = 2 batches x 4 head-groups (4 heads each). Each core
computes QKV projection for its heads, masked attention, and a partial
output projection against its slice of wo's columns; the host sums the 4
partials per batch.

Layout choices (all matmuls bf16 with fp32 PSUM accumulation):
  - Q^T, K^T [dh, S] computed via transposed projection (lhsT = qkv cols).
  - Scores computed transposed: ST[j, q] = K^T.T @ Q^T, so softmax exp runs
    on ACT straight out of PSUM and the PV matmul needs no transposes.
  - V kept token-major [S, dh] with a fused ones-column, so the PV matmul
    (lhsT = [V | 1], rhs = exp(ST)) yields out^T[dh, q] AND the softmax
    denominator row L[q] in one accumulation.
  - Final projection consumes out^T directly as lhsT (no transposes).
  - Mask (window + dilated, depends only on delta = qtile - jtile) is a
    precomputed 0/1 bf16 table multiplied in on DVE after exp.
"""

import sys

sys.path.insert(0, "/opt/trn_rl_repo")

import numpy as np
import ml_dtypes

import concourse.bass as bass
import concourse.mybir as mybir
import concourse.tile as tile
from concourse.vector_clock import ScopedClock
from concourse.bass_utils import run_bass_kernel_spmd

BF16 = mybir.dt.bfloat16
F32 = mybir.dt.float32

B, S, D = 2, 2048, 1024
H, DH = 16, 64
WINDOW, DILATION = 128, 4
P = 128
NT = S // P          # 16 token tiles
KT = D // P          # 8 contraction tiles over D
HPC = 4              # heads per core
QC = 512             # q-chunk width
NQC = S // QC        # 4 q-chunks
NDELTA = 19          # mask table deltas -3..15
NMASKB = 24          # 19 delta masks + Ab [5,128] + Bb [5,512] rank-5 bias
BIAS = 200.0         # additive-mask magnitude: exp(0.125*-200) ~ 1.4e-11


class _TileContext(tile.TileContext):
    """Kernel-tail Drain gets one wait per live proc, but this walrus build
    allows only a single sync wait on SP Drain — split across drains."""

    def _drain_and_barrier(self, tick_clock, wait_clock):
        drain_inst = self.nc.sync.drain()
        wait_clock.add_sem_waits(
            drain_inst.ins, ScopedClock({None: tick_clock.global_clock})
        )
        si = drain_inst.ins.sync_info
        if si is not None and len(si.on_wait) > 1:
            waits = list(si.on_wait)
            si.on_wait[:] = waits[:1]
            for w in waits[1:]:
                d2 = self.nc.sync.drain()
                si2 = d2.ins.sync_info
                if si2 is None:
                    d2.ins.sync_info = mybir.SyncInfo(on_wait=[w], on_update=[])
                else:
                    si2.on_wait[:] = [w]

        self.nc.all_engine_barrier()
        assert self.sems is not None
        popped = self.nc._tile_sem_poison_stack.pop()
        assert popped is self._sem_poison
        self.nc.clear_and_free_semaphores(list(self.sems.allocated().values()))
        self.nc.all_engine_barrier()


def _split_sync_waits(nc):
    """This walrus build allows only one sync-wait slot on several ISA
    structs. Rewrite the scheduled BIR so every instruction carries at most
    one wait: extra waits move onto same-engine NoOps inserted just before
    (same engine queue => executes in order => semantics preserved)."""
    cnt = 0
    for fn in nc.m.functions:
        for blk in fn.blocks:
            new_insts = []
            for inst in blk.instructions:
                si = inst.sync_info
                if si is not None and si.on_wait and len(si.on_wait) > 1:
                    waits = list(si.on_wait)
                    si.on_wait[:] = waits[-1:]
                    for w in waits[:-1]:
                        cnt += 1
                        nop = mybir.InstNoOp(
                            name=f"waitsplit-{cnt}",
                            engine=inst.engine,
                            ins=[],
                            outs=[],
                            sync_info=mybir.SyncInfo(on_wait=[w], on_update=[]),
                        )
                        new_insts.append(nop)
                new_insts.append(inst)
            blk.instructions[:] = new_insts
    return cnt


def _mask_table() -> np.ndarray:
    """[128, NDELTA*128] bf16; block d (delta = d-3) holds mask[sj, sq] for
    score-block ST[jt, qt] with qt - jt = delta: offset = 128*delta + sq - sj."""
    sj = np.arange(P)[:, None]
    sq = np.arange(P)[None, :]
    out = np.zeros((P, NMASKB * P), dtype=ml_dtypes.bfloat16)
    for d in range(NDELTA):
        delta = d - 3
        off = 128 * delta + sq - sj
        valid = ((off >= 0) & (off <= WINDOW - 1)) | (
            (off >= WINDOW + DILATION) & ((off - WINDOW) % DILATION == 0)
        )
        out[:, d * P : (d + 1) * P] = valid.astype(ml_dtypes.bfloat16)
    # rank-5 additive mask for pure-dilated blocks (delta >= 2):
    # valid iff sj % 4 == sq % 4; bias = Ab.T @ Bb = 0 valid / -BIAS invalid.
    r = np.arange(4)[:, None]
    ab = np.zeros((P, P), dtype=ml_dtypes.bfloat16)
    ab[0:4, :] = (BIAS * (np.arange(P)[None, :] % 4 == r)).astype(ml_dtypes.bfloat16)
    ab[4, :] = BIAS
    out[:, NDELTA * P : (NDELTA + 1) * P] = ab
    bb = np.zeros((P, 4 * P), dtype=ml_dtypes.bfloat16)
    bb[0:4, :] = (np.arange(4 * P)[None, :] % 4 == r).astype(ml_dtypes.bfloat16)
    bb[4, :] = -1.0
    out[:, (NDELTA + 1) * P :] = bb
    return out


def _build_program(repeat: int = 1):
    nc = bass.Bass("TRN2", target_bir_lowering=False, debug=False)

    xt_d = nc.declare_dram_parameter("xt", [D, S], BF16, isOutput=False)
    qkvt_d = nc.declare_dram_parameter("qkvt", [D, 3 * HPC * DH], BF16, isOutput=False)
    wot_d = nc.declare_dram_parameter("wot", [HPC * DH, D], BF16, isOutput=False)
    mask_d = nc.declare_dram_parameter("mask", [P, NMASKB * P], BF16, isOutput=False)
    y_d = nc.declare_dram_parameter("y", [S, D], F32, isOutput=True)

    W3 = 3 * HPC * DH  # 768 qkvt columns per k-tile

    with _TileContext(nc) as tc:
        with (
            tc.tile_pool(name="const", bufs=1) as cpool,
            tc.tile_pool(name="work", bufs=2) as wpool,
            tc.tile_pool(name="psum", bufs=2, space="PSUM") as pspool,
        ):
            # ---- persistent SBUF tensors ----
            xt_sb = cpool.tile([P, KT * S], BF16, tag="xt")
            qkvt_sb = cpool.tile([P, KT * W3], BF16, tag="qkvt")
            mask_sb = cpool.tile([P, NMASKB * P], BF16, tag="mask")
            wot_sb = cpool.tile([P, 2 * D], BF16, tag="wot")
            qkt_sb = cpool.tile([P, 4 * S], BF16, tag="qkt")    # QT (mt 0,1), KT (mt 2,3)
            vaug_sb = cpool.tile([P, NT, HPC, DH + 1], BF16, tag="vaug")
            outt_sb = cpool.tile([P, 2 * S], BF16, tag="outt")  # out^T, ct-major
            ones1_sb = cpool.tile([1, 64], mybir.dt.float16, tag="ones1")

            nc.gpsimd.memset(vaug_sb[:], 1.0)
            nc.gpsimd.memset(ones1_sb[:], 1.0)

            for _rep in range(repeat):
                # ---- input DMA ----
                for kt in range(KT):
                    nc.sync.dma_start(
                        out=xt_sb[:, kt * S : (kt + 1) * S],
                        in_=xt_d[kt * P : (kt + 1) * P, :],
                    )
                    nc.sync.dma_start(
                        out=qkvt_sb[:, kt * W3 : (kt + 1) * W3],
                        in_=qkvt_d[kt * P : (kt + 1) * P, :],
                    )
                nc.sync.dma_start(out=mask_sb[:], in_=mask_d[:])
                for ct in range(2):
                    nc.sync.dma_start(
                        out=wot_sb[:, ct * D : (ct + 1) * D],
                        in_=wot_d[ct * P : (ct + 1) * P, :],
                    )

                # ---- QT / KT projection (transposed) ----
                # mt 0,1 = Q channels 0:128 / 128:256; mt 2,3 = K channels.
                for mt in range(4):
                    coloff = (0, 128, 256, 384)[mt]
                    for cc in range(NQC):
                        ps = pspool.tile([P, QC], F32, tag="ps512")
                        for kt in range(KT):
                            nc.tensor.matmul(
                                ps[:],
                                lhsT=qkvt_sb[:, kt * W3 + coloff : kt * W3 + coloff + P],
                                rhs=xt_sb[:, kt * S + cc * QC : kt * S + (cc + 1) * QC],
                                start=(kt == 0),
                                stop=(kt == KT - 1),
                            )
                        nc.scalar.copy(
                            qkt_sb[:, mt * S + cc * QC : mt * S + (cc + 1) * QC], ps[:]
                        )

                # ---- V projection (token-major) into vaug ----
                for nt in range(NT):
                    ps = pspool.tile([P, HPC * DH], F32, tag="ps512")
                    for kt in range(KT):
                        nc.tensor.matmul(
                            ps[:],
                            lhsT=xt_sb[:, kt * S + nt * P : kt * S + (nt + 1) * P],
                            rhs=qkvt_sb[:, kt * W3 + 512 : kt * W3 + 768],
                            start=(kt == 0),
                            stop=(kt == KT - 1),
                        )
                    for h in range(HPC):
                        nc.vector.tensor_copy(
                            vaug_sb[:, nt, h, 0:DH], ps[:, h * DH : (h + 1) * DH]
                        )

                # ---- attention + output projection, per q-chunk ----
                # Normalization is split: phase 1 (no PE) runs right after a
                # head's PV accumulation finishes and frees its PSUM bank by
                # staging poT into SBUF; phase 2 (the ones-matmul broadcast +
                # multiply) is deferred into the NEXT pair's ST stream so the
                # PE never stalls on the reciprocal chain.
                pending = []

                def _flush_pending():
                    while pending:
                        pending.pop(0)()

                def _normalize_p1(h, qc, poT):
                    pb = 64 * (h % 2)
                    qoff = (h // 2) * S
                    po_sb = wpool.tile([64, QC], F32, tag="posb", bufs=4)
                    nc.vector.tensor_copy(po_sb[:], poT[0:64, :])
                    lrow = wpool.tile([1, QC], F32, tag="lrow", bufs=4)
                    nc.scalar.copy(lrow[:], poT[64:65, :])
                    r4 = wpool.tile([P, 4], F32, tag="r4", bufs=4)
                    nc.sync.dma_start(
                        out=r4[:],
                        in_=lrow[0:1, :].rearrange("a (p c) -> a p c", p=P),
                    )
                    i4h = wpool.tile([P, 4], mybir.dt.float16, tag="i4h", bufs=4)
                    with nc.allow_low_precision("softmax 1/L in fp16"):
                        nc.vector.reciprocal(i4h[:], r4[:])
                    invl16 = wpool.tile([1, QC], mybir.dt.float16, tag="invl16", bufs=4)
                    nc.sync.dma_start(
                        out=invl16[0:1, :].rearrange("a (p c) -> a p c", p=P),
                        in_=i4h[:],
                    )

                    def _p2():
                        ib = pspool.tile([64, QC], F32, tag="ps512", bufs=2)
                        nc.tensor.matmul(
                            ib[:], lhsT=ones1_sb[:], rhs=invl16[:], start=True, stop=True
                        )
                        nc.vector.scalar_tensor_tensor(
                            out=outt_sb[
                                pb : pb + 64, qoff + qc * QC : qoff + (qc + 1) * QC
                            ],
                            in0=po_sb[:],
                            scalar=1.0,
                            in1=ib[:],
                            op0=mybir.AluOpType.mult,
                            op1=mybir.AluOpType.mult,
                        )

                    pending.append(_p2)

                def _emit_wo(qc):
                    def _go():
                        for qt in range(4 * qc, 4 * qc + 4):
                            ysb = wpool.tile([P, D], F32, tag="ysb", bufs=2)
                            for oc in range(2):
                                yps = pspool.tile([P, QC], F32, tag="ps512")
                                for ct in range(2):
                                    nc.tensor.matmul(
                                        yps[:],
                                        lhsT=outt_sb[
                                            :, ct * S + qt * P : ct * S + (qt + 1) * P
                                        ],
                                        rhs=wot_sb[
                                            :, ct * D + oc * QC : ct * D + (oc + 1) * QC
                                        ],
                                        start=(ct == 0),
                                        stop=(ct == 1),
                                    )
                                nc.vector.tensor_copy(
                                    ysb[:, oc * QC : (oc + 1) * QC], yps[:]
                                )
                            nc.sync.dma_start(
                                out=y_d[qt * P : (qt + 1) * P, :], in_=ysb[:]
                            )

                    pending.append(_go)

                for qc in range(NQC):
                    njt = 4 * qc + 4
                    # heads processed in pairs (2hp, 2hp+1): their K=64 ST
                    # matmuls sit in disjoint PE row groups (partitions 0-63 /
                    # 64-127) and run concurrently when adjacent in the queue.
                    for hp in range(2):
                        qoff = hp * S
                        koff = (2 + hp) * S
                        poT_a = pspool.tile([P, QC], F32, tag="pot")
                        poT_b = pspool.tile([P, QC], F32, tag="pot")
                        poTs = [poT_a, poT_b]
                        ps_t, e_t = {}, {}

                        def _issue_st(jt):
                            pair = []
                            for i, pb in enumerate((0, 64)):
                                ps = pspool.tile([P, QC], F32, tag="st", bufs=4)
                                nc.tensor.matmul(
                                    ps[:],
                                    lhsT=qkt_sb[
                                        pb : pb + 64,
                                        koff + jt * P : koff + (jt + 1) * P,
                                    ],
                                    rhs=qkt_sb[
                                        pb : pb + 64,
                                        qoff + qc * QC : qoff + (qc + 1) * QC,
                                    ],
                                    start=True,
                                    stop=True,
                                )
                                pair.append(ps)
                            ps_t[jt] = pair

                        def _issue_exp(jt):
                            pure = (4 * qc - jt) >= 2  # all sub-blocks dilated-only
                            didx = 4 * qc - jt + 3
                            pair = []
                            for i in range(2):
                                e = wpool.tile([P, QC], BF16, tag="e", bufs=6)
                                nc.scalar.activation(
                                    e[:],
                                    ps_t[jt][i][:],
                                    mybir.ActivationFunctionType.Exp,
                                    scale=0.125,
                                )
                                # mask multiply: idle GPSIMD for pure-dilated
                                # chunks, DVE for window/edge chunks
                                eng = nc.gpsimd if pure else nc.vector
                                eng.tensor_mul(
                                    e[:], e[:], mask_sb[:, didx * P : didx * P + QC]
                                )
                                pair.append(e)
                            del ps_t[jt]
                            e_t[jt] = pair

                        def _issue_pv(jt):
                            for i in range(2):
                                nc.tensor.matmul(
                                    poTs[i][0:65, :],
                                    lhsT=vaug_sb[:, jt, 2 * hp + i, :],
                                    rhs=e_t[jt][i][:],
                                    start=(jt == 0),
                                    stop=(jt == njt - 1),
                                )
                            del e_t[jt]

                        for step in range(njt + 2):
                            if step == min(4, njt):
                                _flush_pending()
                            if step < njt:
                                _issue_st(step)
                            if 0 <= step - 1 < njt:
                                _issue_exp(step - 1)
                            if 0 <= step - 2 < njt:
                                _issue_pv(step - 2)
                        _normalize_p1(2 * hp, qc, poTs[0])
                        _normalize_p1(2 * hp + 1, qc, poTs[1])
                    _emit_wo(qc)
                _flush_pending()

    _split_sync_waits(nc)
    return nc


_PROGRAMS = {}


def _program(repeat: int = 1):
    if repeat not in _PROGRAMS:
        _PROGRAMS[repeat] = _build_program(repeat)
    return _PROGRAMS[repeat]


def _prep_inputs(x, qkv, wo):
    """Per-core host-side slicing/transposition/casting."""
    mask = _mask_table()
    in_maps = []
    for c in range(8):
        b, hg = c // 4, c % 4
        h0 = HPC * hg
        rows = np.r_[
            h0 * DH : h0 * DH + HPC * DH,
            D + h0 * DH : D + h0 * DH + HPC * DH,
            2 * D + h0 * DH : 2 * D + h0 * DH + HPC * DH,
        ]
        qkvt = np.ascontiguousarray(qkv[rows].T).astype(ml_dtypes.bfloat16)
        xt = np.ascontiguousarray(x[b].T).astype(ml_dtypes.bfloat16)
        wot = np.ascontiguousarray(
            wo[:, h0 * DH : h0 * DH + HPC * DH].T
        ).astype(ml_dtypes.bfloat16)
        in_maps.append({"xt": xt, "qkvt": qkvt, "wot": wot, "mask": mask})
    return in_maps


def kernel(x, qkv, wo, _trace=False, _trace_kwargs=None):
    x = np.asarray(x, dtype=np.float32)
    qkv = np.asarray(qkv, dtype=np.float32)
    wo = np.asarray(wo, dtype=np.float32)

    nc = _program()
    in_maps = _prep_inputs(x, qkv, wo)
    res = run_bass_kernel_spmd(
        nc, in_maps, list(range(8)), trace=_trace, **(_trace_kwargs or {})
    )
    kernel.last_result = res

    y = np.zeros((B, S, D), dtype=np.float32)
    for c in range(8):
        y[c // 4] += res.results[c]["y"]
    return y


# revision 24
# speedup vs baseline: 1.3277x; 1.0335x over previous
"""Trainium2 Bass kernel for CustomWindowMHA (sparse window+dilated attention).

Sharding: 8 cores = 2 batches x 4 head-groups (4 heads each). Each core
computes QKV projection for its heads, masked attention, and a partial
output projection against its slice of wo's columns; the host sums the 4
partials per batch.

Layout choices (all matmuls bf16 with fp32 PSUM accumulation):
  - Q^T, K^T [dh, S] computed via transposed projection (lhsT = qkv cols).
  - Scores computed transposed: ST[j, q] = K^T.T @ Q^T, so softmax exp runs
    on ACT straight out of PSUM and the PV matmul needs no transposes.
  - V kept token-major [S, dh] with a fused ones-column, so the PV matmul
    (lhsT = [V | 1], rhs = exp(ST)) yields out^T[dh, q] AND the softmax
    denominator row L[q] in one accumulation.
  - Final projection consumes out^T directly as lhsT (no transposes).
  - Mask (window + dilated, depends only on delta = qtile - jtile) is a
    precomputed 0/1 bf16 table multiplied in on DVE after exp.
"""

import sys

sys.path.insert(0, "/opt/trn_rl_repo")

import numpy as np
import ml_dtypes

import concourse.bass as bass
import concourse.mybir as mybir
import concourse.tile as tile
from concourse.vector_clock import ScopedClock
from concourse.bass_utils import run_bass_kernel_spmd

BF16 = mybir.dt.bfloat16
F32 = mybir.dt.float32

B, S, D = 2, 2048, 1024
H, DH = 16, 64
WINDOW, DILATION = 128, 4
P = 128
NT = S // P          # 16 token tiles
KT = D // P          # 8 contraction tiles over D
HPC = 4              # heads per core
QC = 512             # q-chunk width
NQC = S // QC        # 4 q-chunks
NDELTA = 19          # mask table deltas -3..15
NMASKB = 24          # 19 delta masks + Ab [5,128] + Bb [5,512] rank-5 bias
BIAS = 200.0         # additive-mask magnitude: exp(0.125*-200) ~ 1.4e-11


class _TileContext(tile.TileContext):
    """Kernel-tail Drain gets one wait per live proc, but this walrus build
    allows only a single sync wait on SP Drain — split across drains."""

    def _drain_and_barrier(self, tick_clock, wait_clock):
        drain_inst = self.nc.sync.drain()
        wait_clock.add_sem_waits(
            drain_inst.ins, ScopedClock({None: tick_clock.global_clock})
        )
        si = drain_inst.ins.sync_info
        if si is not None and len(si.on_wait) > 1:
            waits = list(si.on_wait)
            si.on_wait[:] = waits[:1]
            for w in waits[1:]:
                d2 = self.nc.sync.drain()
                si2 = d2.ins.sync_info
                if si2 is None:
                    d2.ins.sync_info = mybir.SyncInfo(on_wait=[w], on_update=[])
                else:
                    si2.on_wait[:] = [w]

        self.nc.all_engine_barrier()
        assert self.sems is not None
        popped = self.nc._tile_sem_poison_stack.pop()
        assert popped is self._sem_poison
        self.nc.clear_and_free_semaphores(list(self.sems.allocated().values()))
        self.nc.all_engine_barrier()


def _split_sync_waits(nc):
    """This walrus build allows only one sync-wait slot on several ISA
    structs. Rewrite the scheduled BIR so every instruction carries at most
    one wait: extra waits move onto same-engine NoOps inserted just before
    (same engine queue => executes in order => semantics preserved)."""
    cnt = 0
    for fn in nc.m.functions:
        for blk in fn.blocks:
            new_insts = []
            for inst in blk.instructions:
                si = inst.sync_info
                if si is not None and si.on_wait and len(si.on_wait) > 1:
                    waits = list(si.on_wait)
                    si.on_wait[:] = waits[-1:]
                    for w in waits[:-1]:
                        cnt += 1
                        nop = mybir.InstNoOp(
                            name=f"waitsplit-{cnt}",
                            engine=inst.engine,
                            ins=[],
                            outs=[],
                            sync_info=mybir.SyncInfo(on_wait=[w], on_update=[]),
                        )
                        new_insts.append(nop)
                new_insts.append(inst)
            blk.instructions[:] = new_insts
    return cnt


def _mask_table() -> np.ndarray:
    """[128, NDELTA*128] bf16; block d (delta = d-3) holds mask[sj, sq] for
    score-block ST[jt, qt] with qt - jt = delta: offset = 128*delta + sq - sj."""
    sj = np.arange(P)[:, None]
    sq = np.arange(P)[None, :]
    out = np.zeros((P, NMASKB * P), dtype=ml_dtypes.bfloat16)
    for d in range(NDELTA):
        delta = d - 3
        off = 128 * delta + sq - sj
        valid = ((off >= 0) & (off <= WINDOW - 1)) | (
            (off >= WINDOW + DILATION) & ((off - WINDOW) % DILATION == 0)
        )
        out[:, d * P : (d + 1) * P] = valid.astype(ml_dtypes.bfloat16)
    # rank-5 additive mask for pure-dilated blocks (delta >= 2):
    # valid iff sj % 4 == sq % 4; bias = Ab.T @ Bb = 0 valid / -BIAS invalid.
    r = np.arange(4)[:, None]
    ab = np.zeros((P, P), dtype=ml_dtypes.bfloat16)
    ab[0:4, :] = (BIAS * (np.arange(P)[None, :] % 4 == r)).astype(ml_dtypes.bfloat16)
    ab[4, :] = BIAS
    out[:, NDELTA * P : (NDELTA + 1) * P] = ab
    bb = np.zeros((P, 4 * P), dtype=ml_dtypes.bfloat16)
    bb[0:4, :] = (np.arange(4 * P)[None, :] % 4 == r).astype(ml_dtypes.bfloat16)
    bb[4, :] = -1.0
    out[:, (NDELTA + 1) * P :] = bb
    return out


def _build_program(repeat: int = 1):
    nc = bass.Bass("TRN2", target_bir_lowering=False, debug=False)

    xt_d = nc.declare_dram_parameter("xt", [D, S], BF16, isOutput=False)
    qkvt_d = nc.declare_dram_parameter("qkvt", [D, 3 * HPC * DH], BF16, isOutput=False)
    wot_d = nc.declare_dram_parameter("wot", [HPC * DH, D], BF16, isOutput=False)
    mask_d = nc.declare_dram_parameter("mask", [P, NMASKB * P], BF16, isOutput=False)
    y_d = nc.declare_dram_parameter("y", [S, D], F32, isOutput=True)

    W3 = 3 * HPC * DH  # 768 qkvt columns per k-tile

    with _TileContext(nc) as tc:
        with (
            tc.tile_pool(name="const", bufs=1) as cpool,
            tc.tile_pool(name="work", bufs=2) as wpool,
            tc.tile_pool(name="psum", bufs=2, space="PSUM") as pspool,
        ):
            # ---- persistent SBUF tensors ----
            xt_sb = cpool.tile([P, KT * S], BF16, tag="xt")
            qkvt_sb = cpool.tile([P, KT * W3], BF16, tag="qkvt")
            mask_sb = cpool.tile([P, NMASKB * P], BF16, tag="mask")
            wot_sb = cpool.tile([P, 2 * D], BF16, tag="wot")
            qkt_sb = cpool.tile([P, 4 * S], BF16, tag="qkt")    # QT (mt 0,1), KT (mt 2,3)
            vaug_sb = cpool.tile([P, NT, HPC, DH + 1], BF16, tag="vaug")
            outt_sb = cpool.tile([P, 2 * S], BF16, tag="outt")  # out^T, ct-major
            ones1_sb = cpool.tile([1, 64], mybir.dt.float16, tag="ones1")

            nc.gpsimd.memset(vaug_sb[:], 1.0)
            nc.gpsimd.memset(ones1_sb[:], 1.0)

            for _rep in range(repeat):
                # ---- input DMA ----
                for kt in range(KT):
                    nc.sync.dma_start(
                        out=xt_sb[:, kt * S : (kt + 1) * S],
                        in_=xt_d[kt * P : (kt + 1) * P, :],
                    )
                    nc.sync.dma_start(
                        out=qkvt_sb[:, kt * W3 : (kt + 1) * W3],
                        in_=qkvt_d[kt * P : (kt + 1) * P, :],
                    )
                nc.sync.dma_start(out=mask_sb[:], in_=mask_d[:])
                for ct in range(2):
                    nc.sync.dma_start(
                        out=wot_sb[:, ct * D : (ct + 1) * D],
                        in_=wot_d[ct * P : (ct + 1) * P, :],
                    )

                # ---- QT / KT projection (transposed) ----
                # mt 0,1 = Q channels 0:128 / 128:256; mt 2,3 = K channels.
                for mt in range(4):
                    coloff = (0, 128, 256, 384)[mt]
                    for cc in range(NQC):
                        ps = pspool.tile([P, QC], F32, tag="ps512")
                        for kt in range(KT):
                            nc.tensor.matmul(
                                ps[:],
                                lhsT=qkvt_sb[:, kt * W3 + coloff : kt * W3 + coloff + P],
                                rhs=xt_sb[:, kt * S + cc * QC : kt * S + (cc + 1) * QC],
                                start=(kt == 0),
                                stop=(kt == KT - 1),
                            )
                        nc.scalar.copy(
                            qkt_sb[:, mt * S + cc * QC : mt * S + (cc + 1) * QC], ps[:]
                        )

                # ---- V projection (token-major) into vaug ----
                for nt in range(NT):
                    ps = pspool.tile([P, HPC * DH], F32, tag="ps512")
                    for kt in range(KT):
                        nc.tensor.matmul(
                            ps[:],
                            lhsT=xt_sb[:, kt * S + nt * P : kt * S + (nt + 1) * P],
                            rhs=qkvt_sb[:, kt * W3 + 512 : kt * W3 + 768],
                            start=(kt == 0),
                            stop=(kt == KT - 1),
                        )
                    for h in range(HPC):
                        nc.vector.tensor_copy(
                            vaug_sb[:, nt, h, 0:DH], ps[:, h * DH : (h + 1) * DH]
                        )

                # ---- attention + output projection, per q-chunk ----
                # Normalization is split: phase 1 (no PE) runs right after a
                # head's PV accumulation finishes and frees its PSUM bank by
                # staging poT into SBUF; phase 2 (the ones-matmul broadcast +
                # multiply) is deferred into the NEXT pair's ST stream so the
                # PE never stalls on the reciprocal chain.
                pending = []

                def _flush_pending():
                    while pending:
                        pending.pop(0)()

                def _normalize_p1(h, qc, poT):
                    pb = 64 * (h % 2)
                    qoff = (h // 2) * S
                    po_sb = wpool.tile([64, QC], F32, tag="posb", bufs=4)
                    nc.vector.tensor_copy(po_sb[:], poT[0:64, :])
                    lrow = wpool.tile([1, QC], F32, tag="lrow", bufs=4)
                    nc.scalar.copy(lrow[:], poT[64:65, :])
                    r4 = wpool.tile([P, 4], F32, tag="r4", bufs=4)
                    nc.sync.dma_start(
                        out=r4[:],
                        in_=lrow[0:1, :].rearrange("a (p c) -> a p c", p=P),
                    )
                    i4h = wpool.tile([P, 4], mybir.dt.float16, tag="i4h", bufs=4)
                    with nc.allow_low_precision("softmax 1/L in fp16"):
                        nc.vector.reciprocal(i4h[:], r4[:])
                    invl16 = wpool.tile([1, QC], mybir.dt.float16, tag="invl16", bufs=4)
                    nc.sync.dma_start(
                        out=invl16[0:1, :].rearrange("a (p c) -> a p c", p=P),
                        in_=i4h[:],
                    )

                    def _p2():
                        ib = pspool.tile([64, QC], F32, tag="ps512", bufs=2)
                        nc.tensor.matmul(
                            ib[:], lhsT=ones1_sb[:], rhs=invl16[:], start=True, stop=True
                        )
                        nc.vector.scalar_tensor_tensor(
                            out=outt_sb[
                                pb : pb + 64, qoff + qc * QC : qoff + (qc + 1) * QC
                            ],
                            in0=po_sb[:],
                            scalar=1.0,
                            in1=ib[:],
                            op0=mybir.AluOpType.mult,
                            op1=mybir.AluOpType.mult,
                        )

                    pending.append(_p2)

                def _emit_wo(qc):
                    def _go():
                        for qt in range(4 * qc, 4 * qc + 4):
                            ysb = wpool.tile([P, D], F32, tag="ysb", bufs=2)
                            for oc in range(2):
                                yps = pspool.tile([P, QC], F32, tag="ps512")
                                for ct in range(2):
                                    nc.tensor.matmul(
                                        yps[:],
                                        lhsT=outt_sb[
                                            :, ct * S + qt * P : ct * S + (qt + 1) * P
                                        ],
                                        rhs=wot_sb[
                                            :, ct * D + oc * QC : ct * D + (oc + 1) * QC
                                        ],
                                        start=(ct == 0),
                                        stop=(ct == 1),
                                    )
                                nc.vector.tensor_copy(
                                    ysb[:, oc * QC : (oc + 1) * QC], yps[:]
                                )
                            nc.sync.dma_start(
                                out=y_d[qt * P : (qt + 1) * P, :], in_=ysb[:]
                            )

                    pending.append(_go)

                for qc in range(NQC):
                    njt = 4 * qc + 4
                    # heads processed in pairs (2hp, 2hp+1): their K=64 ST
                    # matmuls sit in disjoint PE row groups (partitions 0-63 /
                    # 64-127) and run concurrently when adjacent in the queue.
                    for hp in range(2):
                        qoff = hp * S
                        koff = (2 + hp) * S
                        poT_a = pspool.tile([P, QC], F32, tag="pot")
                        poT_b = pspool.tile([P, QC], F32, tag="pot")
                        poTs = [poT_a, poT_b]
                        ps_t, e_t = {}, {}

                        def _issue_st(jt):
                            # columns q < 128*jt are above the causal diagonal:
                            # skip whole fully-masked leading 128-blocks
                            off = max(0, (jt - 4 * qc)) * P
                            pair = []
                            for i, pb in enumerate((0, 64)):
                                ps = pspool.tile([P, QC], F32, tag="st", bufs=4)
                                nc.tensor.matmul(
                                    ps[:, off:],
                                    lhsT=qkt_sb[
                                        pb : pb + 64,
                                        koff + jt * P : koff + (jt + 1) * P,
                                    ],
                                    rhs=qkt_sb[
                                        pb : pb + 64,
                                        qoff + qc * QC + off : qoff + (qc + 1) * QC,
                                    ],
                                    start=True,
                                    stop=True,
                                )
                                pair.append(ps)
                            ps_t[jt] = pair

                        def _issue_exp(jt):
                            pure = (4 * qc - jt) >= 2  # all sub-blocks dilated-only
                            off = max(0, (jt - 4 * qc)) * P
                            didx = 4 * qc - jt + 3 + off // P
                            pair = []
                            for i in range(2):
                                e = wpool.tile([P, QC], BF16, tag="e", bufs=6)
                                nc.scalar.activation(
                                    e[:, off:],
                                    ps_t[jt][i][:, off:],
                                    mybir.ActivationFunctionType.Exp,
                                    scale=0.125,
                                )
                                # mask multiply: idle GPSIMD for pure-dilated
                                # chunks, DVE for window/edge chunks
                                eng = nc.gpsimd if pure else nc.vector
                                eng.tensor_mul(
                                    e[:, off:],
                                    e[:, off:],
                                    mask_sb[:, didx * P : didx * P + QC - off],
                                )
                                pair.append(e)
                            del ps_t[jt]
                            e_t[jt] = pair

                        def _issue_pv(jt):
                            off = max(0, (jt - 4 * qc)) * P
                            for i in range(2):
                                nc.tensor.matmul(
                                    poTs[i][0:65, off:],
                                    lhsT=vaug_sb[:, jt, 2 * hp + i, :],
                                    rhs=e_t[jt][i][:, off:],
                                    start=(jt == 0),
                                    stop=(jt == njt - 1),
                                )
                            del e_t[jt]

                        for step in range(njt + 2):
                            if step == min(4, njt):
                                _flush_pending()
                            if step < njt:
                                _issue_st(step)
                            if 0 <= step - 1 < njt:
                                _issue_exp(step - 1)
                            if 0 <= step - 2 < njt:
                                _issue_pv(step - 2)
                        _normalize_p1(2 * hp, qc, poTs[0])
                        _normalize_p1(2 * hp + 1, qc, poTs[1])
                    _emit_wo(qc)
                _flush_pending()

    _split_sync_waits(nc)
    return nc


_PROGRAMS = {}


def _program(repeat: int = 1):
    if repeat not in _PROGRAMS:
        _PROGRAMS[repeat] = _build_program(repeat)
    return _PROGRAMS[repeat]


def _prep_inputs(x, qkv, wo):
    """Per-core host-side slicing/transposition/casting."""
    mask = _mask_table()
    in_maps = []
    for c in range(8):
        b, hg = c // 4, c % 4
        h0 = HPC * hg
        rows = np.r_[
            h0 * DH : h0 * DH + HPC * DH,
            D + h0 * DH : D + h0 * DH + HPC * DH,
            2 * D + h0 * DH : 2 * D + h0 * DH + HPC * DH,
        ]
        qkvt = np.ascontiguousarray(qkv[rows].T).astype(ml_dtypes.bfloat16)
        xt = np.ascontiguousarray(x[b].T).astype(ml_dtypes.bfloat16)
        wot = np.ascontiguousarray(
            wo[:, h0 * DH : h0 * DH + HPC * DH].T
        ).astype(ml_dtypes.bfloat16)
        in_maps.append({"xt": xt, "qkvt": qkvt, "wot": wot, "mask": mask})
    return in_maps


def kernel(x, qkv, wo, _trace=False, _trace_kwargs=None):
    x = np.asarray(x, dtype=np.float32)
    qkv = np.asarray(qkv, dtype=np.float32)
    wo = np.asarray(wo, dtype=np.float32)

    nc = _program()
    in_maps = _prep_inputs(x, qkv, wo)
    res = run_bass_kernel_spmd(
        nc, in_maps, list(range(8)), trace=_trace, **(_trace_kwargs or {})
    )
    kernel.last_result = res

    y = np.zeros((B, S, D), dtype=np.float32)
    for c in range(8):
        y[c // 4] += res.results[c]["y"]
    return y


# revision 25
# speedup vs baseline: 1.3804x; 1.0397x over previous
"""Trainium2 Bass kernel for CustomWindowMHA (sparse window+dilated attention).

Sharding: 8 cores = 2 batches x 4 head-groups (4 heads each). Each core
computes QKV projection for its heads, masked attention, and a partial
output projection against its slice of wo's columns; the host sums the 4
partials per batch.

Layout choices (all matmuls bf16 with fp32 PSUM accumulation):
  - Q^T, K^T [dh, S] computed via transposed projection (lhsT = qkv cols).
  - Scores computed transposed: ST[j, q] = K^T.T @ Q^T, so softmax exp runs
    on ACT straight out of PSUM and the PV matmul needs no transposes.
  - V kept token-major [S, dh] with a fused ones-column, so the PV matmul
    (lhsT = [V | 1], rhs = exp(ST)) yields out^T[dh, q] AND the softmax
    denominator row L[q] in one accumulation.
  - Final projection consumes out^T directly as lhsT (no transposes).
  - Mask (window + dilated, depends only on delta = qtile - jtile) is a
    precomputed 0/1 bf16 table multiplied in on DVE after exp.
"""

import sys

sys.path.insert(0, "/opt/trn_rl_repo")

import numpy as np
import ml_dtypes

import concourse.bass as bass
import concourse.mybir as mybir
import concourse.tile as tile
from concourse.vector_clock import ScopedClock
from concourse.bass_utils import run_bass_kernel_spmd

BF16 = mybir.dt.bfloat16
F32 = mybir.dt.float32

B, S, D = 2, 2048, 1024
H, DH = 16, 64
WINDOW, DILATION = 128, 4
P = 128
NT = S // P          # 16 token tiles
KT = D // P          # 8 contraction tiles over D
HPC = 4              # heads per core
QC = 512             # q-chunk width
NQC = S // QC        # 4 q-chunks
NDELTA = 19          # mask table deltas -3..15
NMASKB = 24          # 19 delta masks + Ab [5,128] + Bb [5,512] rank-5 bias
BIAS = 200.0         # additive-mask magnitude: exp(0.125*-200) ~ 1.4e-11


class _TileContext(tile.TileContext):
    """Kernel-tail Drain gets one wait per live proc, but this walrus build
    allows only a single sync wait on SP Drain — split across drains."""

    def _drain_and_barrier(self, tick_clock, wait_clock):
        drain_inst = self.nc.sync.drain()
        wait_clock.add_sem_waits(
            drain_inst.ins, ScopedClock({None: tick_clock.global_clock})
        )
        si = drain_inst.ins.sync_info
        if si is not None and len(si.on_wait) > 1:
            waits = list(si.on_wait)
            si.on_wait[:] = waits[:1]
            for w in waits[1:]:
                d2 = self.nc.sync.drain()
                si2 = d2.ins.sync_info
                if si2 is None:
                    d2.ins.sync_info = mybir.SyncInfo(on_wait=[w], on_update=[])
                else:
                    si2.on_wait[:] = [w]

        self.nc.all_engine_barrier()
        assert self.sems is not None
        popped = self.nc._tile_sem_poison_stack.pop()
        assert popped is self._sem_poison
        self.nc.clear_and_free_semaphores(list(self.sems.allocated().values()))
        self.nc.all_engine_barrier()


def _split_sync_waits(nc):
    """This walrus build allows only one sync-wait slot on several ISA
    structs. Rewrite the scheduled BIR so every instruction carries at most
    one wait: extra waits move onto same-engine NoOps inserted just before
    (same engine queue => executes in order => semantics preserved)."""
    cnt = 0
    for fn in nc.m.functions:
        for blk in fn.blocks:
            new_insts = []
            for inst in blk.instructions:
                si = inst.sync_info
                if si is not None and si.on_wait and len(si.on_wait) > 1:
                    waits = list(si.on_wait)
                    si.on_wait[:] = waits[-1:]
                    for w in waits[:-1]:
                        cnt += 1
                        nop = mybir.InstNoOp(
                            name=f"waitsplit-{cnt}",
                            engine=inst.engine,
                            ins=[],
                            outs=[],
                            sync_info=mybir.SyncInfo(on_wait=[w], on_update=[]),
                        )
                        new_insts.append(nop)
                new_insts.append(inst)
            blk.instructions[:] = new_insts
    return cnt


def _mask_table() -> np.ndarray:
    """[128, NDELTA*128] bf16; block d (delta = d-3) holds mask[sj, sq] for
    score-block ST[jt, qt] with qt - jt = delta: offset = 128*delta + sq - sj."""
    sj = np.arange(P)[:, None]
    sq = np.arange(P)[None, :]
    out = np.zeros((P, NMASKB * P), dtype=ml_dtypes.bfloat16)
    for d in range(NDELTA):
        delta = d - 3
        off = 128 * delta + sq - sj
        valid = ((off >= 0) & (off <= WINDOW - 1)) | (
            (off >= WINDOW + DILATION) & ((off - WINDOW) % DILATION == 0)
        )
        out[:, d * P : (d + 1) * P] = valid.astype(ml_dtypes.bfloat16)
    # rank-5 additive mask for pure-dilated blocks (delta >= 2):
    # valid iff sj % 4 == sq % 4; bias = Ab.T @ Bb = 0 valid / -BIAS invalid.
    r = np.arange(4)[:, None]
    ab = np.zeros((P, P), dtype=ml_dtypes.bfloat16)
    ab[0:4, :] = (BIAS * (np.arange(P)[None, :] % 4 == r)).astype(ml_dtypes.bfloat16)
    ab[4, :] = BIAS
    out[:, NDELTA * P : (NDELTA + 1) * P] = ab
    bb = np.zeros((P, 4 * P), dtype=ml_dtypes.bfloat16)
    bb[0:4, :] = (np.arange(4 * P)[None, :] % 4 == r).astype(ml_dtypes.bfloat16)
    bb[4, :] = -1.0
    out[:, (NDELTA + 1) * P :] = bb
    return out


def _build_program(repeat: int = 1):
    nc = bass.Bass("TRN2", target_bir_lowering=False, debug=False)

    xt_d = nc.declare_dram_parameter("xt", [D, S], BF16, isOutput=False)
    qkvt_d = nc.declare_dram_parameter("qkvt", [D, 3 * HPC * DH], BF16, isOutput=False)
    wot_d = nc.declare_dram_parameter("wot", [HPC * DH, D], BF16, isOutput=False)
    mask_d = nc.declare_dram_parameter("mask", [P, NMASKB * P], BF16, isOutput=False)
    y_d = nc.declare_dram_parameter("y", [S, D], F32, isOutput=True)

    W3 = 3 * HPC * DH  # 768 qkvt columns per k-tile

    with _TileContext(nc) as tc:
        with (
            tc.tile_pool(name="const", bufs=1) as cpool,
            tc.tile_pool(name="work", bufs=2) as wpool,
            tc.tile_pool(name="psum", bufs=2, space="PSUM") as pspool,
        ):
            # ---- persistent SBUF tensors ----
            xt_sb = cpool.tile([P, KT * S], BF16, tag="xt")
            qkvt_sb = cpool.tile([P, KT * W3], BF16, tag="qkvt")
            mask_sb = cpool.tile([P, NMASKB * P], BF16, tag="mask")
            wot_sb = cpool.tile([P, 2 * D], BF16, tag="wot")
            qkt_sb = cpool.tile([P, 4 * S], BF16, tag="qkt")    # QT (mt 0,1), KT (mt 2,3)
            vaug_sb = cpool.tile([P, NT, HPC, DH + 1], BF16, tag="vaug")
            outt_sb = cpool.tile([P, 2 * S], BF16, tag="outt")  # out^T, ct-major
            ones1_sb = cpool.tile([1, 64], mybir.dt.float16, tag="ones1")

            nc.gpsimd.memset(vaug_sb[:], 1.0)
            nc.gpsimd.memset(ones1_sb[:], 1.0)

            for _rep in range(repeat):
                # ---- input DMA ----
                for kt in range(KT):
                    nc.sync.dma_start(
                        out=xt_sb[:, kt * S : (kt + 1) * S],
                        in_=xt_d[kt * P : (kt + 1) * P, :],
                    )
                    nc.sync.dma_start(
                        out=qkvt_sb[:, kt * W3 : (kt + 1) * W3],
                        in_=qkvt_d[kt * P : (kt + 1) * P, :],
                    )
                nc.sync.dma_start(out=mask_sb[:], in_=mask_d[:])
                for ct in range(2):
                    nc.sync.dma_start(
                        out=wot_sb[:, ct * D : (ct + 1) * D],
                        in_=wot_d[ct * P : (ct + 1) * P, :],
                    )

                # ---- QT / KT projection (transposed) ----
                # mt 0,1 = Q channels 0:128 / 128:256; mt 2,3 = K channels.
                for mt in range(4):
                    coloff = (0, 128, 256, 384)[mt]
                    for cc in range(NQC):
                        ps = pspool.tile([P, QC], F32, tag="ps512")
                        for kt in range(KT):
                            nc.tensor.matmul(
                                ps[:],
                                lhsT=qkvt_sb[:, kt * W3 + coloff : kt * W3 + coloff + P],
                                rhs=xt_sb[:, kt * S + cc * QC : kt * S + (cc + 1) * QC],
                                start=(kt == 0),
                                stop=(kt == KT - 1),
                            )
                        nc.scalar.copy(
                            qkt_sb[:, mt * S + cc * QC : mt * S + (cc + 1) * QC], ps[:]
                        )

                # ---- V projection (token-major) into vaug ----
                for nt in range(NT):
                    ps = pspool.tile([P, HPC * DH], F32, tag="ps512")
                    for kt in range(KT):
                        nc.tensor.matmul(
                            ps[:],
                            lhsT=xt_sb[:, kt * S + nt * P : kt * S + (nt + 1) * P],
                            rhs=qkvt_sb[:, kt * W3 + 512 : kt * W3 + 768],
                            start=(kt == 0),
                            stop=(kt == KT - 1),
                        )
                    for h in range(HPC):
                        nc.vector.tensor_copy(
                            vaug_sb[:, nt, h, 0:DH], ps[:, h * DH : (h + 1) * DH]
                        )

                # ---- attention + output projection, per q-chunk ----
                # Normalization is split: phase 1 (no PE) runs right after a
                # head's PV accumulation finishes and frees its PSUM bank by
                # staging poT into SBUF; phase 2 (the ones-matmul broadcast +
                # multiply) is deferred into the NEXT pair's ST stream so the
                # PE never stalls on the reciprocal chain.
                pending = []

                def _flush_pending():
                    while pending:
                        pending.pop(0)()

                def _normalize_p1(h, qc, poT):
                    pb = 64 * (h % 2)
                    qoff = (h // 2) * S
                    po_sb = wpool.tile([64, QC], F32, tag="posb", bufs=4)
                    nc.vector.tensor_copy(po_sb[:], poT[0:64, :])
                    lrow = wpool.tile([1, QC], F32, tag="lrow", bufs=4)
                    nc.scalar.copy(lrow[:], poT[64:65, :])
                    r4 = wpool.tile([P, 4], F32, tag="r4", bufs=4)
                    nc.sync.dma_start(
                        out=r4[:],
                        in_=lrow[0:1, :].rearrange("a (p c) -> a p c", p=P),
                    )
                    i4h = wpool.tile([P, 4], mybir.dt.float16, tag="i4h", bufs=4)
                    with nc.allow_low_precision("softmax 1/L in fp16"):
                        nc.vector.reciprocal(i4h[:], r4[:])
                    invl16 = wpool.tile([1, QC], mybir.dt.float16, tag="invl16", bufs=4)
                    nc.sync.dma_start(
                        out=invl16[0:1, :].rearrange("a (p c) -> a p c", p=P),
                        in_=i4h[:],
                    )

                    def _p2():
                        ib = pspool.tile([64, QC], F32, tag="ps512", bufs=2)
                        nc.tensor.matmul(
                            ib[:], lhsT=ones1_sb[:], rhs=invl16[:], start=True, stop=True
                        )
                        nc.vector.scalar_tensor_tensor(
                            out=outt_sb[
                                pb : pb + 64, qoff + qc * QC : qoff + (qc + 1) * QC
                            ],
                            in0=po_sb[:],
                            scalar=1.0,
                            in1=ib[:],
                            op0=mybir.AluOpType.mult,
                            op1=mybir.AluOpType.mult,
                        )

                    pending.append(_p2)

                def _emit_wo(qc):
                    def _go():
                        for qt in range(4 * qc, 4 * qc + 4):
                            ysb = wpool.tile([P, D], F32, tag="ysb", bufs=2)
                            for oc in range(2):
                                yps = pspool.tile([P, QC], F32, tag="ps512")
                                for ct in range(2):
                                    nc.tensor.matmul(
                                        yps[:],
                                        lhsT=outt_sb[
                                            :, ct * S + qt * P : ct * S + (qt + 1) * P
                                        ],
                                        rhs=wot_sb[
                                            :, ct * D + oc * QC : ct * D + (oc + 1) * QC
                                        ],
                                        start=(ct == 0),
                                        stop=(ct == 1),
                                    )
                                nc.vector.tensor_copy(
                                    ysb[:, oc * QC : (oc + 1) * QC], yps[:]
                                )
                            nc.sync.dma_start(
                                out=y_d[qt * P : (qt + 1) * P, :], in_=ysb[:]
                            )

                    pending.append(_go)

                for qc in range(NQC):
                    njt = 4 * qc + 4
                    # heads processed in pairs (2hp, 2hp+1): their K=64 ST
                    # matmuls sit in disjoint PE row groups (partitions 0-63 /
                    # 64-127) and run concurrently when adjacent in the queue.
                    for hp in range(2):
                        qoff = hp * S
                        koff = (2 + hp) * S
                        poT_a = pspool.tile([P, QC], F32, tag="pot")
                        poT_b = pspool.tile([P, QC], F32, tag="pot")
                        poTs = [poT_a, poT_b]
                        ps_t, e_t = {}, {}

                        def _issue_st(jt):
                            # columns q < 128*jt are above the causal diagonal:
                            # skip whole fully-masked leading 128-blocks
                            off = max(0, (jt - 4 * qc)) * P
                            pair = []
                            for i, pb in enumerate((0, 64)):
                                ps = pspool.tile([P, QC], F32, tag="st", bufs=4)
                                nc.tensor.matmul(
                                    ps[:, off:],
                                    lhsT=qkt_sb[
                                        pb : pb + 64,
                                        koff + jt * P : koff + (jt + 1) * P,
                                    ],
                                    rhs=qkt_sb[
                                        pb : pb + 64,
                                        qoff + qc * QC + off : qoff + (qc + 1) * QC,
                                    ],
                                    start=True,
                                    stop=True,
                                )
                                pair.append(ps)
                            ps_t[jt] = pair

                        def _issue_exp(jt):
                            pure = (4 * qc - jt) >= 2  # all sub-blocks dilated-only
                            off = max(0, (jt - 4 * qc)) * P
                            didx = 4 * qc - jt + 3 + off // P
                            pair = []
                            for i in range(2):
                                e = wpool.tile([P, QC], BF16, tag="e", bufs=8)
                                nc.scalar.activation(
                                    e[:, off:],
                                    ps_t[jt][i][:, off:],
                                    mybir.ActivationFunctionType.Exp,
                                    scale=0.125,
                                )
                                # mask multiply: idle GPSIMD for pure-dilated
                                # chunks, DVE for window/edge chunks
                                eng = nc.gpsimd if pure else nc.vector
                                eng.tensor_mul(
                                    e[:, off:],
                                    e[:, off:],
                                    mask_sb[:, didx * P : didx * P + QC - off],
                                )
                                pair.append(e)
                            del ps_t[jt]
                            e_t[jt] = pair

                        def _issue_pv(jt):
                            off = max(0, (jt - 4 * qc)) * P
                            for i in range(2):
                                nc.tensor.matmul(
                                    poTs[i][0:65, off:],
                                    lhsT=vaug_sb[:, jt, 2 * hp + i, :],
                                    rhs=e_t[jt][i][:, off:],
                                    start=(jt == 0),
                                    stop=(jt == njt - 1),
                                )
                            del e_t[jt]

                        for step in range(njt + 2):
                            if step == min(4, njt):
                                _flush_pending()
                            if step < njt:
                                _issue_st(step)
                            if 0 <= step - 1 < njt:
                                _issue_exp(step - 1)
                            if 0 <= step - 2 < njt:
                                _issue_pv(step - 2)
                        _normalize_p1(2 * hp, qc, poTs[0])
                        _normalize_p1(2 * hp + 1, qc, poTs[1])
                    _emit_wo(qc)
                _flush_pending()

    _split_sync_waits(nc)
    return nc


_PROGRAMS = {}


def _program(repeat: int = 1):
    if repeat not in _PROGRAMS:
        _PROGRAMS[repeat] = _build_program(repeat)
    return _PROGRAMS[repeat]


def _prep_inputs(x, qkv, wo):
    """Per-core host-side slicing/transposition/casting."""
    mask = _mask_table()
    in_maps = []
    for c in range(8):
        b, hg = c // 4, c % 4
        h0 = HPC * hg
        rows = np.r_[
            h0 * DH : h0 * DH + HPC * DH,
            D + h0 * DH : D + h0 * DH + HPC * DH,
            2 * D + h0 * DH : 2 * D + h0 * DH + HPC * DH,
        ]
        qkvt = np.ascontiguousarray(qkv[rows].T).astype(ml_dtypes.bfloat16)
        xt = np.ascontiguousarray(x[b].T).astype(ml_dtypes.bfloat16)
        wot = np.ascontiguousarray(
            wo[:, h0 * DH : h0 * DH + HPC * DH].T
        ).astype(ml_dtypes.bfloat16)
        in_maps.append({"xt": xt, "qkvt": qkvt, "wot": wot, "mask": mask})
    return in_maps


def kernel(x, qkv, wo, _trace=False, _trace_kwargs=None):
    x = np.asarray(x, dtype=np.float32)
    qkv = np.asarray(qkv, dtype=np.float32)
    wo = np.asarray(wo, dtype=np.float32)

    nc = _program()
    in_maps = _prep_inputs(x, qkv, wo)
    res = run_bass_kernel_spmd(
        nc, in_maps, list(range(8)), trace=_trace, **(_trace_kwargs or {})
    )
    kernel.last_result = res

    y = np.zeros((B, S, D), dtype=np.float32)
    for c in range(8):
        y[c // 4] += res.results[c]["y"]
    return y
